# revision 9
# baseline (speedup 1.0000x reference)
"""GCN layer (linear + weighted scatter-add aggregation) on 8 TRN2 NeuronCores.

Reference computation:
    h = x @ W.T                      [N, D]
    out[r] = sum_{e: row[e]==r} val[e] * h[col[e]]

Key identity: the linear layer commutes past the (linear) aggregation:
    out = (A @ x) @ W.T    where A[r,c] = sum of val over edges (r,c)
so we aggregate raw x rows first (8x less matmul work, no h materialization).

Distribution: destination nodes are sharded 12500/core (edges partitioned by
destination so the segment-sum is fully local; x is replicated to each core's
HBM by the host, so no collective is needed).

Per-core algorithm ("rank-window form"):
  - Destinations are packed into per-call rank slots (RC=160 ranks/call,
    8 windows of W_R=20 ranks). A call covers 4096 edge slots: 4 source
    chunks x 8 windows x 128 slots. Chunk windows overlap (int16-indexable
    ranges starting at 0/22500/45000/67500) so boundary edges can be
    assigned to either chunk, balancing the 4 chunk streams exactly.
  - Four batched GPSIMD dma_gather calls per call (1024 int16 indices each)
    pull x rows (bf16, 512B each) into SBUF as [128 slots, 32 groups, 256].
  - A banded scaled one-hot S ([128, 32, 20], S[p,g,r] = val * (seg==r)) is
    built with 2 batched DVE ops per call.
  - PE: per (window j, feat-half h): 4 chunk matmuls accumulate
    aggT[feat, 20j:20j+20] += xg_group^T @ S_group into a [128, 2*160] f32
    PSUM tile (gathered rows are the *stationary* operand; the moving free
    dim is the 20-wide rank band, so matmul cost ~ rank width, not feature
    width, and the aggregate comes out transposed - no PE transposes).
  - aggT -> SBUF bf16, then 4 matmuls against W.T (bf16, f32 PSUM) produce
    the 160 output rows per call. Destinations split across calls/ranks are
    re-merged on the host (np.add.at).
"""

import os
import sys

sys.path.insert(0, "/opt/trn_rl_repo")
os.environ.setdefault("MYCRO_LOCAL_CACHE", "1")

from collections import deque
from contextlib import ExitStack

import numpy as np
import ml_dtypes

import concourse.bass as bass
import concourse.bacc as bacc
import concourse.mybir as mybir
import concourse.tile as tile
from concourse.bass_utils import run_bass_kernel_spmd
from concourse.library_config import mlp as _mlp_lib

N_NODES = 100000
N_CORES = 8
NPC = N_NODES // N_CORES  # dests per core
D = 256
SLOTS = 128  # edge slots per group (= matmul K)
NCHUNK = 4
CHUNK_BASE = [0, 22500, 45000, 67500]  # overlapping int16-indexable windows
CHUNK_END = [32768, 55268, 77768, 100000]
W_R = 24  # ranks per window (= S band width)
NWIN = 8  # windows per call
RC = NWIN * W_R  # 160 rank slots per call
CG = NCHUNK * NWIN  # 32 groups per call
CALL_SLOTS = CG * SLOTS  # 4096 edge slots per call
GATHER_IDX = NWIN * SLOTS  # 1024 indices per (call, chunk) dma_gather
GM = 8  # meta calls grouped per DMA (fewer HWDGE setups)
HB = NWIN // 2  # windows per output half
RH = HB * W_R  # ranks per output half

BF16 = ml_dtypes.bfloat16


# ----------------------------------------------------------------------------
# Host-side packing
# ----------------------------------------------------------------------------

def assign_chunks(cols, n_calls_cap=None):
    """Assign each edge to a chunk, balancing the 4 per-chunk edge counts
    via the overlap regions. Returns (chunk_id, local_idx)."""
    n = len(cols)
    chunk = np.full(n, -1, np.int8)
    # exclusive regions
    chunk[cols < CHUNK_BASE[1]] = 0
    chunk[(cols >= CHUNK_END[0]) & (cols < CHUNK_BASE[2])] = 1
    chunk[(cols >= CHUNK_END[1]) & (cols < CHUNK_BASE[3])] = 2
    chunk[cols >= CHUNK_END[2]] = 3
    target = -(-n // 4)
    counts = [int(np.sum(chunk == c)) for c in range(4)]
    for c in range(3):
        # overlap between chunk c and c+1
        ov = np.nonzero((chunk == -1) & (cols >= CHUNK_BASE[c + 1]) & (cols < CHUNK_END[c]))[0]
        take = min(len(ov), max(0, target - counts[c]))
        if take:
            chunk[ov[:take]] = c
            counts[c] += take
        if len(ov) > take:
            chunk[ov[take:]] = c + 1
            counts[c + 1] += len(ov) - take
    assert np.all(chunk >= 0)
    base = np.asarray(CHUNK_BASE, np.int64)[chunk]
    loc = cols - base
    assert loc.min() >= 0 and loc.max() < 32768
    return chunk.astype(np.int64), loc.astype(np.int64)


def pack_core(rows_loc, cols, vals, npc, variant=0):
    """Pack one core's edges (dest-local ids in [0, npc)) into calls.

    Each call has RC=160 rank slots in 8 windows of 20; window j of chunk c
    is one 128-slot group. Dests (items) are fed greedily; an item's edges
    in chunk c go into group (c, j) of its window; whatever doesn't fit is
    carried to the next call (the dest re-enters under a new rank and the
    partial sums are re-combined on the host).

    Returns flat idx/seg/val slot arrays plus per-item (vrow, dest).
    """
    chunk_id, loc = assign_chunks(cols)
    key = rows_loc.astype(np.int64) * NCHUNK + chunk_id
    order = np.argsort(key, kind="stable")
    loc_s = loc[order]
    vals_s = vals[order]
    dc_deg = np.bincount(key, minlength=npc * NCHUNK).astype(np.int64)
    dc_deg = dc_deg.reshape(npc, NCHUNK)
    dc_start = np.zeros(npc * NCHUNK + 1, np.int64)
    dc_start[1:] = np.cumsum(dc_deg.ravel())
    dc_start = dc_start[:-1].reshape(npc, NCHUNK)
    deg = dc_deg.sum(1)

    # Exact window filling needs clipping; clip only BIG items so carries
    # re-enter as useful near-fresh items (tiny carries eat rank slots).
    # Feed order is a search dimension (pack_all retries variants until a
    # core hits its call floor): 0 = big/small alternating, 1 = descending,
    # >=2 = seeded shuffle.
    asc = np.argsort(deg, kind="stable")
    if variant == 0:
        srt = np.empty_like(asc)
        srt[0::2] = asc[::-1][: (npc + 1) // 2]
        srt[1::2] = asc[: npc // 2]
    elif variant == 1:
        srt = asc[::-1]
    else:
        rng = np.random.default_rng(variant - 1)
        srt = asc.copy()
        rng.shuffle(srt)
    bigq = deque(
        (int(d), dc_deg[int(d)].copy(), np.zeros(NCHUNK, np.int64))
        for d in srt
        if deg[d] > 0
    )
    carryq = deque()

    items_dest, items_call, items_rank = [], [], []
    items_take, items_coff, items_qoff = [], [], []
    call = 0
    while bigq or carryq:
        for j in range(NWIN):
            used = np.zeros(NCHUNK, np.int64)
            nrank = 0
            stuck = []
            while nrank < W_R and not np.all(used >= SLOTS):
                cap = SLOTS - used
                it = None
                clip = False
                # a fully-fitting item first: carries, then big, then small
                if carryq and np.all(carryq[0][1] <= cap):
                    it = carryq.popleft()
                elif bigq and np.all(bigq[0][1] <= cap):
                    it = bigq.popleft()
                elif bigq and np.all(bigq[-1][1] <= cap):
                    it = bigq.pop()
                elif bigq:
                    it = bigq.popleft()
                    clip = True
                elif carryq:
                    it = carryq.popleft()
                    clip = True
                else:
                    break
                d, rem, coff = it
                take = np.minimum(rem, cap) if clip else rem
                if take.sum() == 0:
                    stuck.append((d, rem, coff))
                    continue
                items_dest.append(d)
                items_call.append(call)
                items_rank.append(j * W_R + nrank)
                items_take.append(take.copy())
                items_coff.append(coff.copy())
                items_qoff.append(used.copy())
                used += take
                nrank += 1
                rem = rem - take
                if rem.sum() > 0:
                    carryq.append((d, rem, coff + take))
            carryq.extendleft(reversed(stuck))
        call += 1
    n_calls = call

    n_items = len(items_dest)
    items_dest = np.asarray(items_dest, np.int64)
    items_call = np.asarray(items_call, np.int64)
    items_rank = np.asarray(items_rank, np.int64)
    items_take = np.asarray(items_take, np.int64)  # [n_items, NCHUNK]
    items_coff = np.asarray(items_coff, np.int64)
    items_qoff = np.asarray(items_qoff, np.int64)

    # expand per-(item, chunk) runs into slot positions
    win = items_rank // W_R
    flat_deg = items_take.ravel()
    cgrid = np.tile(np.arange(NCHUNK), n_items)
    irep = np.repeat(np.arange(n_items), NCHUNK)
    e_start = (dc_start[items_dest] + items_coff).ravel()
    # slot position: call*4096 + (8c + j)*128 + qoff
    slot_base = (
        items_call[irep] * CALL_SLOTS
        + (NWIN * cgrid + win[irep]) * SLOTS
        + items_qoff.ravel()
    )
    nz = np.nonzero(flat_deg)[0]
    nz_deg = flat_deg[nz]
    reps = np.repeat(np.arange(len(nz)), nz_deg)
    csum = np.zeros(len(nz) + 1, np.int64)
    csum[1:] = np.cumsum(nz_deg)
    within = np.arange(int(nz_deg.sum()), dtype=np.int64) - csum[reps]
    e_pos = e_start[nz][reps] + within
    slot = slot_base[nz][reps] + within

    idx_slot = np.zeros(n_calls * CALL_SLOTS, np.int32)
    val_slot = np.zeros(n_calls * CALL_SLOTS, np.float32)
    seg_slot = np.zeros(n_calls * CALL_SLOTS, np.int16)
    idx_slot[slot] = loc_s[e_pos]
    val_slot[slot] = vals_s[e_pos]
    seg_slot[slot] = (items_rank % W_R)[irep[nz]][reps]

    vrow = items_call * RC + items_rank
    # windows actually populated in the final call (for tail trimming)
    last_mask = items_call == n_calls - 1
    nwu_last = int((items_rank[last_mask] // W_R).max()) + 1 if last_mask.any() else 0
    return dict(
        n_calls=n_calls,
        nwu_last=nwu_last,
        idx=idx_slot,
        val=val_slot,
        seg=seg_slot,
        vrow=vrow,
        dest=items_dest,
        n_edges=len(rows_loc),
    )


def pack_all(edge_row, edge_col, edge_val, n_nodes=N_NODES, n_cores=N_CORES):
    npc = n_nodes // n_cores
    core_id = edge_row // npc
    packs = []
    for i in range(n_cores):
        m = core_id == i
        # the perfect floor is rarely reachable (interior padding); settle
        # for floor+1 to bound the variant search
        floor_calls = -(-int(m.sum()) // CALL_SLOTS) + 1
        p = None
        for variant in range(6):
            q = pack_core(
                edge_row[m] - i * npc, edge_col[m], edge_val[m], npc, variant
            )
            if p is None or q["n_calls"] < p["n_calls"]:
                p = q
            if p["n_calls"] <= floor_calls:
                break
        packs.append(p)
    return packs


def build_call_arrays(p, n_calls):
    """DRAM layout: one int16 meta tensor [n_calls, 128, 4*64+32+32]:
    4 chunk-gather index blocks (wrapped in 16 partitions and replicated
    across the 8 gpsimd cores), then seg, then val (bf16 bit-packed).

    The dma_gather for (call, chunk c) consumes the call's 8 chunk-c groups
    in order; index position i -> (partition i%128, group i//128), wrapped
    so position i sits at [i%16, i//16] (replicated over each 16-partition
    block).
    """
    gtot = n_calls * CG

    def lay(a, np_dtype):
        full = np.zeros(gtot * SLOTS, a.dtype)
        full[: len(a)] = a
        return np.ascontiguousarray(
            full.reshape(n_calls, CG, SLOTS).transpose(0, 2, 1)
        ).astype(np_dtype)

    idx_full = np.zeros(gtot * SLOTS, np.int64)
    idx_full[: len(p["idx"])] = p["idx"]
    byg = idx_full.reshape(n_calls, CG, SLOTS)
    iw = GATHER_IDX // 16
    meta = np.empty((n_calls, 128, NCHUNK * iw + 2 * CG), np.int16)
    for c in range(NCHUNK):
        flat = byg[:, NWIN * c : NWIN * (c + 1), :].reshape(n_calls, GATHER_IDX)
        wrapped = flat.reshape(n_calls, iw, 16).transpose(0, 2, 1)
        meta[:, :, c * iw : (c + 1) * iw] = np.tile(wrapped, (1, 8, 1)).astype(
            np.int16
        )
    o = NCHUNK * iw
    meta[:, :, o : o + CG] = lay(p["seg"], BF16).view(np.int16)
    meta[:, :, o + CG : o + 2 * CG] = lay(p["val"], BF16).view(np.int16)
    # group GM calls per meta row-block: one DMA loads GM calls' meta
    ng = -(-n_calls // GM)
    mw = meta.shape[2]
    grouped = np.zeros((ng * GM, 128, mw), np.int16)
    grouped[:n_calls] = meta
    grouped = np.ascontiguousarray(
        grouped.reshape(ng, GM, 128, mw).transpose(0, 2, 1, 3).reshape(ng, 128, GM * mw)
    )
    return grouped


# ----------------------------------------------------------------------------
# Device program
# ----------------------------------------------------------------------------

def build_program(n_calls, nwu_last=NWIN, n_nodes=N_NODES, d=D):
    nc = bacc.Bacc("TRN2", target_bir_lowering=False, debug=False)
    f32 = mybir.dt.float32
    bf16 = mybir.dt.bfloat16

    x = nc.dram_tensor("xb", [n_nodes, d], bf16, kind="ExternalInput")
    iw = GATHER_IDX // 16  # idx words per chunk-gather per partition
    mw = NCHUNK * iw + 2 * CG  # meta words per partition per call
    ng = -(-n_calls // GM)
    metaT = nc.dram_tensor(
        "meta", [ng, 128, GM * mw], mybir.dt.int16, kind="ExternalInput"
    )
    wtT = nc.dram_tensor("wt", [d // 128, 128, d], bf16, kind="ExternalInput")
    iotaT = nc.dram_tensor("iota", [128, W_R], bf16, kind="ExternalInput")
    out = nc.dram_tensor("out", [n_calls * RC, d], bf16, kind="ExternalOutput")

    kh = d // 128  # feature half-tiles

    with tile.TileContext(nc) as tc, ExitStack() as ctx:
        const = ctx.enter_context(tc.tile_pool(name="const", bufs=1))
        sb = ctx.enter_context(tc.tile_pool(name="sb", bufs=6))
        xgp = ctx.enter_context(tc.tile_pool(name="xg", bufs=4))
        ps = ctx.enter_context(tc.tile_pool(name="ps", bufs=2, space="PSUM"))

        nc.gpsimd.load_library(_mlp_lib)

        # prefetch the first meta group ahead of the const loads so the
        # first gather's indices are ready as early as possible (HWDGE
        # setups serialize; consts aren't needed until the first W-matmul)
        mt = sb.tile([128, GM * mw], mybir.dt.int16, tag="meta")
        nc.sync.dma_start(mt[:], metaT[0])
        iota_t = const.tile([128, W_R], bf16)
        nc.sync.dma_start(iota_t[:], iotaT[:, :])
        wt_t = const.tile([128, kh * d], bf16)
        for h in range(kh):
            nc.sync.dma_start(wt_t[:, h * d : (h + 1) * d], wtT[h])
        for cl in range(n_calls):
            nwu = nwu_last if cl == n_calls - 1 else NWIN
            gidx = nwu * SLOTS  # indices per chunk-gather this call
            if cl % GM == 0 and cl > 0:
                mt = sb.tile([128, GM * mw], mybir.dt.int16, tag="meta")
                nc.sync.dma_start(mt[:], metaT[cl // GM])
            mo = (cl % GM) * mw
            idx_t = mt[:, mo : mo + mw]

            xg = xgp.tile([SLOTS, CG, d], bf16, tag="xg")
            # the last call's gathers are split into window-halves so the
            # first half's aggregation/output overlaps the second half's
            # transfers (shortens the end-of-kernel drain)
            halves = (
                [(0, min(nwu, HB)), (HB, nwu)] if cl == n_calls - 1 else [(0, nwu)]
            )
            for wlo, whi in halves:
                if whi <= wlo:
                    continue
                gx = (whi - wlo) * SLOTS
                for c in range(NCHUNK):
                    lo = CHUNK_BASE[c]
                    hi = CHUNK_END[c]
                    nc.gpsimd.dma_gather(
                        xg[:, NWIN * c + wlo : NWIN * c + whi, :],
                        x[lo:hi, :],
                        idx_t[:, c * iw + wlo * SLOTS // 16 : c * iw + whi * SLOTS // 16],
                        gx,
                        gx,
                        d,
                    )

            o = NCHUNK * iw
            seg_t = idx_t[:, o : o + CG].bitcast(bf16)
            val_t = idx_t[:, o + CG : o + 2 * CG].bitcast(bf16)

            # banded scaled one-hot: S[p, g, r] = val[p,g] * (seg[p,g] == r)
            d1 = sb.tile([SLOTS, CG, W_R], bf16, tag="d1")
            nc.vector.tensor_tensor(
                out=d1[:],
                in0=seg_t.unsqueeze(2).to_broadcast([SLOTS, CG, W_R]),
                in1=iota_t[:].unsqueeze(1).to_broadcast([SLOTS, CG, W_R]),
                op=mybir.AluOpType.subtract,
            )
            s_t = sb.tile([SLOTS, CG, W_R], bf16, tag="s")
            nc.vector.scalar_tensor_tensor(
                out=s_t[:],
                in0=d1[:],
                scalar=0.0,
                op0=mybir.AluOpType.is_equal,
                in1=val_t.unsqueeze(2).to_broadcast([SLOTS, CG, W_R]),
                op1=mybir.AluOpType.mult,
            )

            # transposed aggregate: aggT[feat_h, h*RC + rank] in f32 PSUM,
            # processed in two window-halves so the first half's output
            # stage (copy/W-matmul/store) overlaps the second half's
            # aggregation - this shortens the end-of-kernel drain
            rows = nwu * W_R
            pagg = ps.tile([128, kh, RC], f32, tag="pagg")
            for half in range(2):
                jlo, jhi = half * HB, min(nwu, (half + 1) * HB)
                if jhi <= jlo:
                    continue
                for j in range(jlo, jhi):
                    for h in range(kh):
                        ro = j * W_R
                        for c in range(NCHUNK):
                            g = NWIN * c + j
                            nc.tensor.matmul(
                                out=pagg[:, h, ro : ro + W_R],
                                lhsT=xg[:, g, h * 128 : (h + 1) * 128],
                                rhs=s_t[:, g, :],
                                start=(c == 0),
                                stop=(c == NCHUNK - 1),
                            )
                rh = min(rows - half * RH, RH)  # rows in this half
                aggs = sb.tile([128, kh, RH], bf16, tag=f"aggs{half}")
                nc.vector.tensor_copy(
                    out=aggs[:],
                    in_=pagg[:, :, half * RH : (half + 1) * RH],
                )
                pout = ps.tile([RH, d], f32, tag=f"pout{half}")
                for h in range(kh):
                    nc.tensor.matmul(
                        out=pout[0:rh, :],
                        lhsT=aggs[:, h, 0:rh],
                        rhs=wt_t[:, h * d : (h + 1) * d],
                        start=(h == 0),
                        stop=(h == kh - 1),
                    )
                osb = sb.tile([RH, d], bf16, tag=f"osb{half}")
                nc.vector.tensor_copy(out=osb[0:rh, :], in_=pout[0:rh, :])
                nc.scalar.dma_start(
                    out[cl * RC + half * RH : cl * RC + half * RH + rh, :],
                    osb[0:rh, :],
                )

    nc.compile()
    return nc


# ----------------------------------------------------------------------------
# Entry point
# ----------------------------------------------------------------------------

_PROG_CACHE = {}


def _get_program(n_calls, nwu_last=NWIN):
    key = (n_calls, nwu_last)
    if key not in _PROG_CACHE:
        _PROG_CACHE[key] = build_program(n_calls, nwu_last)
    return _PROG_CACHE[key]


def make_in_maps(x, W, packs, n_calls):
    xb = np.ascontiguousarray(x.astype(BF16))
    wt = np.ascontiguousarray(W.T.reshape(D // 128, 128, D).astype(BF16))
    iota = np.broadcast_to(np.arange(W_R, dtype=np.float32), (128, W_R))
    iota = np.ascontiguousarray(iota.astype(BF16))
    in_maps = []
    for p in packs:
        meta = build_call_arrays(p, n_calls)
        in_maps.append(dict(xb=xb, meta=meta, wt=wt, iota=iota))
    return in_maps


def kernel(x, W, edge_val, edge_row, edge_col, _return_results=False, trace=False):
    packs = pack_all(edge_row, edge_col, edge_val)
    n_calls = max(p["n_calls"] for p in packs)
    nwu_last = max(
        p["nwu_last"] if p["n_calls"] == n_calls else NWIN for p in packs
    )
    nc = _get_program(n_calls, nwu_last)
    in_maps = make_in_maps(x, W, packs, n_calls)
    res = run_bass_kernel_spmd(
        nc, in_maps, core_ids=list(range(N_CORES)), trace=trace
    )
    out = np.zeros((N_NODES, D), np.float32)
    for i, (p, core_out) in enumerate(zip(packs, res.results)):
        ov = np.asarray(core_out["out"]).astype(np.float32)
        true_ids = p["dest"] + i * NPC
        np.add.at(out, true_ids, ov[p["vrow"]])
    if _return_results:
        return out, res
    return out
